# revision 5
# baseline (speedup 1.0000x reference)
"""Multi-head causal linear attention (B=1, N=2048, D=1024, H=16) on 8 trn2 cores.

Math: reference computes, per head (e=64):
    q = softmax(q_raw, -1) * e**-0.5 ;  k = exp(k_raw)
    out_n = (q_n . KV_n) / (q_n . (kcum_n + EPS)),  KV_n = sum_{j<=n} k_j v_j^T
Because both numerator and denominator are linear in q_n, the softmax
normalization and the e**-0.5 scale cancel exactly; only u = exp(q_raw)
matters.  The EPS term contributes <1e-6 relative and is dropped.  The
v-bias contribution factors out:  out += b_v  (sum_j s_nj / denom ~= 1).

Per-core work (head-parallel, 2 heads/core):
  * k-outer projection: per 128-wide d-slice, q/k feature-major matmuls
    (W stationary, x^T moving) and v token-major (x^T chunk stationary,
    Wv moving) so each k-slice's matmuls start as soon as its [W | x]
    DMA lands -- the input stream paces the first token tile instead of
    stalling the PE behind a monolithic transfer.
  * Chunked causal linear attention (chunk=128): intra (masked QK^T V)
    + inter (running KV state) recurrence. The KV state is kept
    block-diagonal over the 2 heads so inter and delta are single
    matmuls over both heads.
"""

import os
from contextlib import ExitStack

import numpy as np

import concourse.bass as bass
import concourse.mybir as mybir
import concourse.tile as tile
from concourse import bacc
from concourse._compat import with_exitstack
from concourse.bass import ts

FP32 = mybir.dt.float32
BF16 = mybir.dt.bfloat16

B, N, D, H = 1, 2048, 1024, 16
E = D // H          # 64 head dim
NCORES = 8
HPC = H // NCORES   # 2 heads per core
KT = D // 128       # 8 contraction (d) slices
TT = 512            # token tile (projection granularity)
NTT = N // TT       # 4
C = 128             # chunk (tokens) for the causal recurrence
CPT = TT // C       # 4 chunks per token tile
NC = N // C         # 16 chunks total
WB = 3 * 128        # W columns per k-slice (q|k|v blocks)
KSL = WB + TT       # mega cols per k-slice: [Wq|Wk|Wv|X0]
NJUNK = 8           # PE warm-up matmuls (bridge DMA arm latency ~3.4us)

Exp = mybir.ActivationFunctionType.Exp
MULT = mybir.AluOpType.mult
ADD = mybir.AluOpType.add


@with_exitstack
def _emit(ctx: ExitStack, tc, io):
    nc = tc.nc
    mega_d, cf_d, msk_d, xt1_d, xt23_d, out_d = io

    const = ctx.enter_context(tc.tile_pool(name="const", bufs=1))
    chain = ctx.enter_context(tc.tile_pool(name="chain", bufs=2))
    smtp = ctx.enter_context(tc.tile_pool(name="smtp", bufs=4))
    small = ctx.enter_context(tc.tile_pool(name="small", bufs=3))
    outp = ctx.enter_context(tc.tile_pool(name="outp", bufs=3))
    pproj = ctx.enter_context(tc.tile_pool(name="pproj", bufs=1, space="PSUM"))
    ps_s = ctx.enter_context(tc.tile_pool(name="ps_s", bufs=2, space="PSUM"))
    ps_tr = ctx.enter_context(tc.tile_pool(name="ps_tr", bufs=1, space="PSUM"))
    ps_od = ctx.enter_context(tc.tile_pool(name="ps_od", bufs=2, space="PSUM"))

    # ---- persistent SBUF ----
    # mega: [ ident 128 | (Wq|Wk|Wv 384 | xt0 512) x 8 ]
    mega_sb = const.tile([128, 128 + KT * KSL], BF16)
    cf_sb = const.tile([128, 2 + HPC * E], FP32)   # [bq|bk|bv]
    msk_sb = const.tile([128, C], BF16)            # causal mask [j, i], 1 iff j<=i
    xtr_sb = const.tile([128, (NTT - 1) * KT * TT], BF16)  # xt tt=1..3, (tt k t)

    id_sb = mega_sb[:, 0:128]
    bq_sb = cf_sb[:, 0:1]
    bk_sb = cf_sb[:, 1:2]
    bv_sb = cf_sb[:, 2 : 2 + HPC * E]

    def w_ap(k, f):
        base = 128 + k * KSL + f * 128
        return mega_sb[:, base : base + 128]

    def xt_ap(tt, k):
        if tt == 0:
            base = 128 + k * KSL + WB
            return mega_sb[:, base : base + TT]
        base = (tt - 1) * KT * TT + k * TT
        return xtr_sb[:, base : base + TT]

    # PE warm-up bridge: junk matmuls on zeros keep the HAM activity window
    # busy while inputs stream in, so real matmuls start at 2.4 GHz.
    scratch = const.tile([128, TT], BF16)
    nc.gpsimd.memset(scratch[:, :], 0.0)
    junk_ps = ps_s.tile([128, TT], FP32, tag="s", name="junk")
    for j in range(NJUNK):
        nc.tensor.matmul(
            junk_ps[:, :],
            lhsT=scratch[:, 0:128],
            rhs=scratch[:, :],
            start=True,
            stop=True,
        )

    # input DMAs, strictly in consumption order on one engine so the HBM
    # stream is never stolen by data needed later. k-slice granularity
    # gates the tile-0 projection per-slice.
    M0 = 128  # ident rides with slice 0
    nc.sync.dma_start(mega_sb[:, 0 : M0 + KSL], mega_d[:, 0 : M0 + KSL])
    for k in range(1, 4):
        sl = slice(M0 + k * KSL, M0 + (k + 1) * KSL)
        nc.sync.dma_start(mega_sb[:, sl], mega_d[:, sl])
    nc.sync.dma_start(cf_sb[:, :], cf_d[:, :])
    nc.sync.dma_start(msk_sb[:, :], msk_d[:, :])
    for k in range(4, KT):
        sl = slice(M0 + k * KSL, M0 + (k + 1) * KSL)
        nc.sync.dma_start(mega_sb[:, sl], mega_d[:, sl])
    HT = KT * TT // 2
    nc.sync.dma_start(xtr_sb[:, 0:HT], xt1_d[:, 0:HT])
    nc.sync.dma_start(xtr_sb[:, HT : 2 * HT], xt1_d[:, HT : 2 * HT])
    nc.sync.dma_start(xtr_sb[:, 2 * HT : 4 * HT], xt23_d[:, 0 : 2 * HT])
    nc.sync.dma_start(xtr_sb[:, 4 * HT : 6 * HT], xt23_d[:, 2 * HT : 4 * HT])

    # running KV state, head h on partitions [64h:64h+64]
    kv_prev = None   # bf16 block-diagonal [128, HPC, E+1] (matmul operand)
    kv_f32 = None    # fp32 master accumulator [128, E+1]
    dma_flip = [0]

    st = [dict() for _ in range(NTT)]

    def emit_proj_slice(tt, k):
        # per d-slice: q/k feature-major (W stationary), v token-major
        # (x chunk stationary) so v lands [token, feat] and needs no
        # transpose before the delta matmul.
        s = st[tt]
        if k == 0:
            s["qps"] = pproj.tile([128, TT], FP32, tag="q", name=f"qps{tt}")
            s["kps"] = pproj.tile([128, TT], FP32, tag="k", name=f"kps{tt}")
            s["vps"] = pproj.tile([128, CPT, C], FP32, tag="v", name=f"vps{tt}")
        first, last = k == 0, k == KT - 1
        nc.tensor.matmul(
            s["kps"][:, :], lhsT=w_ap(k, 1), rhs=xt_ap(tt, k),
            start=first, stop=last,
        )
        nc.tensor.matmul(
            s["qps"][:, :], lhsT=w_ap(k, 0), rhs=xt_ap(tt, k),
            start=first, stop=last,
        )
        for cc in range(CPT):
            nc.tensor.matmul(
                s["vps"][:, cc, :],
                lhsT=xt_ap(tt, k)[:, ts(cc, C)],
                rhs=w_ap(k, 2),
                start=(first and cc == 0),
                stop=(last and cc == CPT - 1),
            )

    def emit_activations(tt):
        s = st[tt]
        s["EkT"] = EkT = chain.tile([128, TT], BF16, tag="EkT", name=f"EkT{tt}")
        nc.scalar.activation(EkT[:, :], s["kps"][:, :], Exp, bias=bk_sb[:, 0:1])
        s["UT"] = UT = chain.tile([128, TT], BF16, tag="UT", name=f"UT{tt}")
        nc.scalar.activation(UT[:, :], s["qps"][:, :], Exp, bias=bq_sb[:, 0:1])
        s["v_augs"] = []
        for cc in range(CPT):
            v_aug = small.tile(
                [128, HPC, E + 1], BF16, tag="vaug", bufs=6, name=f"vaug{tt}_{cc}"
            )
            nc.scalar.copy(
                v_aug[:, :, 0:E],
                s["vps"][:, cc, :].rearrange("p (g e) -> p g e", g=HPC),
            )
            nc.gpsimd.memset(v_aug[:, :, E : E + 1], 1.0)
            s["v_augs"].append(v_aug)

    def emit_prep(tt):
        # token-layout Ek (PE transpose + ACT copy), chunk scores, masked
        s = st[tt]
        UT, EkT = s["UT"], s["EkT"]
        trp = ps_tr.tile([128, CPT, C], BF16, tag="tr", name=f"trp{tt}")
        s["ek_toks"] = []
        s["smt"] = []
        for cc in range(CPT):
            nc.tensor.transpose(trp[:, cc, :], EkT[:, ts(cc, C)], id_sb[:, :])
            ek_tok = small.tile(
                [128, 128], BF16, tag="ektok", bufs=6, name=f"ektok{tt}_{cc}"
            )
            nc.scalar.copy(ek_tok[:, :], trp[:, cc, :])
            s["ek_toks"].append(ek_tok)
        sps = [
            ps_s.tile([128, TT], FP32, tag="s", name=f"sp{tt}_{h}")
            for h in range(HPC)
        ]
        for cc in range(CPT):
            for h in range(HPC):
                nc.tensor.matmul(
                    sps[h][:, ts(cc, C)],
                    lhsT=EkT[ts(h, E), ts(cc, C)],
                    rhs=UT[ts(h, E), ts(cc, C)],
                    start=True,
                    stop=True,
                    tile_position=(E * h, 0),
                )
            # per-chunk masked scores so chunk 0 can start while later
            # chunks' S matmuls are still in flight
            for h in range(HPC):
                sm = smtp.tile(
                    [128, C], BF16, tag=f"smt{h}", name=f"smt{tt}_{cc}_{h}"
                )
                nc.vector.tensor_mul(sm[:, :], sps[h][:, ts(cc, C)], msk_sb[:, :])
                s["smt"].append(sm)

    def finalize(out_ps, osb, ftt, fcc):
        fc = ftt * CPT + fcc
        last_chunk = fc == NC - 1
        rec = small.tile([128, HPC], FP32, tag="rec", name=f"rec{fc}")
        nc.vector.reciprocal(rec[:, :], out_ps[:, :, E])
        for h in range(HPC):
            nc.vector.scalar_tensor_tensor(
                osb[:, fcc, ts(h, E)],
                in0=out_ps[:, h, 0:E],
                scalar=rec[:, h : h + 1],
                in1=bv_sb[:, ts(h, E)],
                op0=MULT,
                op1=ADD,
            )
            if last_chunk:
                # very last chunk: ship each head half right after its own
                # stt, on separate trigger engines, so the end-of-kernel
                # drain waits on a smaller, earlier transfer
                eng2 = nc.sync if h == 0 else nc.gpsimd
                eng2.dma_start(
                    out_d[ts(fc, C), ts(h, E)], osb[:, fcc, ts(h, E)]
                )
        if last_chunk:
            return
        eng = nc.gpsimd if dma_flip[0] % 2 else nc.sync
        if ftt == NTT - 1:
            # last token tile: ship each chunk as soon as it's done, on
            # alternating trigger engines, to shorten the kernel tail
            dma_flip[0] += 1
            eng.dma_start(out_d[ts(fc, C), :], osb[:, fcc, :])
        elif fcc == CPT - 1:
            dma_flip[0] += 1
            eng.dma_start(
                out_d[ts(ftt, TT), :].rearrange("(cc p) f -> p cc f", p=128),
                osb[:, :, :],
            )

    osb = None

    def emit_chain_chunk(tt, cc):
        nonlocal kv_prev, kv_f32, osb
        s = st[tt]
        UT = s["UT"]
        c = tt * CPT + cc
        ek_tok = s["ek_toks"][cc]
        v_aug = s["v_augs"][cc]
        od = ps_od.tile([128, 2 * HPC, E + 1], FP32, tag="od", name=f"od{c}")
        out_ps = od[:, 0:HPC, :]
        delta_ps = od[:, HPC : 2 * HPC, :]
        # intra: per head (stationary = masked scores); inter: one matmul
        # over both heads against the block-diagonal KV state
        for h in range(HPC):
            nc.tensor.matmul(
                out_ps[:, h, :],
                lhsT=s["smt"][cc * HPC + h][:, :],
                rhs=v_aug[:, h, :],
                start=(h == 0),
                stop=(c == 0 and h == HPC - 1),
            )
        if c > 0:
            nc.tensor.matmul(
                out_ps[:, :, :],
                lhsT=UT[:, ts(cc, C)],
                rhs=kv_prev[:, :, :],
                start=False,
                stop=True,
            )
        if c < NC - 1:
            nc.tensor.matmul(
                delta_ps[:, :, :],
                lhsT=ek_tok[:, :],
                rhs=v_aug[:, :, :],
                start=True,
                stop=True,
            )
            kv_bf = small.tile(
                [128, HPC, E + 1], BF16, tag="kv", bufs=2, name=f"kvb{c}"
            )
            kv_new = small.tile([128, E + 1], FP32, tag="kvm", name=f"kvm{c}")
            for h in range(HPC):
                hp = ts(h, E)
                # off-diagonal block must be zero for the merged inter
                nc.gpsimd.memset(kv_bf[hp, 1 - h, :], 0.0)
                if c == 0:
                    nc.vector.tensor_copy(kv_bf[hp, h, :], delta_ps[hp, h, :])
                    nc.vector.tensor_copy(kv_new[hp, :], delta_ps[hp, h, :])
                else:
                    nc.vector.tensor_add(
                        kv_bf[hp, h, :], delta_ps[hp, h, :], kv_f32[hp, :]
                    )
                    nc.vector.tensor_add(
                        kv_new[hp, :], delta_ps[hp, h, :], kv_f32[hp, :]
                    )
            kv_prev, kv_f32 = kv_bf, kv_new

        if cc == 0:
            osb = outp.tile([128, CPT, HPC * E], FP32, tag="osb", name=f"osb{tt}")
        finalize(out_ps, osb, tt, cc)

    # ---- software-pipelined emission: the next tile's projection k-slices
    # are woven between the chain chunks so the PE's in-order stream always
    # has independent work queued behind each cross-engine dependency of
    # the sequential state chain.
    for k in range(KT):
        emit_proj_slice(0, k)
    emit_activations(0)
    emit_prep(0)
    for tt in range(NTT):
        if tt < NTT - 1:
            nxt = tt + 1
            ks = [(0, 1, 2), (3, 4, 5), (6, 7)]

            def mk(ki):
                def go():
                    for k in ks[ki]:
                        emit_proj_slice(nxt, k)
                    if ki == 2:
                        emit_activations(nxt)
                        emit_prep(nxt)
                return go

            slices = [mk(0), mk(1), mk(2), None]
        else:
            slices = [None] * CPT
        for cc in range(CPT):
            emit_chain_chunk(tt, cc)
            if slices[cc] is not None:
                slices[cc]()


def build_nc():
    nc = bacc.Bacc(
        "TRN2",
        target_bir_lowering=False,
        debug=False,
        enable_asserts=False,
        num_devices=NCORES,
    )
    mega_d = nc.dram_tensor(
        "mega", [128, 128 + KT * KSL], BF16, kind="ExternalInput"
    ).ap()
    cf_d = nc.dram_tensor(
        "cf", [128, 2 + HPC * E], FP32, kind="ExternalInput"
    ).ap()
    msk_d = nc.dram_tensor("msk", [128, C], BF16, kind="ExternalInput").ap()
    xt1_d = nc.dram_tensor("xt1", [128, KT * TT], BF16, kind="ExternalInput").ap()
    xt23_d = nc.dram_tensor(
        "xt23", [128, 2 * KT * TT], BF16, kind="ExternalInput"
    ).ap()
    out_d = nc.dram_tensor("out", [N, HPC * E], FP32, kind="ExternalOutput").ap()
    io = (mega_d, cf_d, msk_d, xt1_d, xt23_d, out_d)
    with tile.TileContext(nc) as tc:
        _emit(tc, io)
    nc.compile()
    return nc


def host_inputs(x, W_qvk, b_qvk):
    """Full inputs -> per-core in_maps (host-side shard + transpose)."""
    import ml_dtypes

    x = np.asarray(x, dtype=np.float32).reshape(N, D)
    W = np.asarray(W_qvk, dtype=np.float32)
    b = np.asarray(b_qvk, dtype=np.float32)
    xt = x.T.astype(ml_dtypes.bfloat16)  # (D, N)

    def pack(a):  # (D, M) -> (128, KT*M), partition-contiguous
        kt, m = a.shape[0] // 128, a.shape[1]
        return np.ascontiguousarray(
            a.reshape(kt, 128, m).transpose(1, 0, 2).reshape(128, kt * m)
        )

    xtp = [pack(xt[:, tt * TT : (tt + 1) * TT]) for tt in range(NTT)]
    xt1 = xtp[1]
    xt23 = np.ascontiguousarray(np.concatenate([xtp[2], xtp[3]], axis=1))
    ident = np.eye(128, dtype=ml_dtypes.bfloat16)

    tri = np.tril(np.ones((C, C), dtype=np.float32))  # [i, j] valid j<=i
    mask = np.ascontiguousarray(tri.T.astype(ml_dtypes.bfloat16))  # [j, i]

    in_maps = []
    for core in range(NCORES):
        heads = [HPC * core + i for i in range(HPC)]
        # torch.chunk order in reference: q, v, k
        qcols = np.concatenate([np.arange(E * h, E * h + E) for h in heads])
        vcols = qcols + D
        kcols = qcols + 2 * D
        Wc = np.concatenate(
            [W[:, qcols], W[:, kcols], W[:, vcols]], axis=1
        ).astype(ml_dtypes.bfloat16)  # (D, 384)
        # mega: [ident | (Wq|Wk|Wv | xt0_k) x 8]
        parts = [ident]
        for k in range(KT):
            parts.append(Wc[k * 128 : (k + 1) * 128, :])
            parts.append(xtp[0][:, k * TT : (k + 1) * TT])
        mega = np.ascontiguousarray(np.concatenate(parts, axis=1))
        bq = b[qcols].reshape(128, 1)
        bk = b[kcols].reshape(128, 1)
        bv = np.broadcast_to(b[vcols], (128, HPC * E))
        cf = np.ascontiguousarray(
            np.concatenate([bq, bk, bv], axis=1, dtype=np.float32)
        )
        in_maps.append(dict(mega=mega, cf=cf, msk=mask, xt1=xt1, xt23=xt23))
    return in_maps


_CACHE = {}


def kernel(x, W_qvk, b_qvk, head_num):
    assert int(np.asarray(head_num)) == H
    if "nc" not in _CACHE:
        _CACHE["nc"] = build_nc()
    nc = _CACHE["nc"]
    in_maps = host_inputs(x, W_qvk, b_qvk)
    from concourse.bass_utils import run_bass_kernel_spmd

    res = run_bass_kernel_spmd(
        nc,
        in_maps,
        core_ids=list(range(NCORES)),
        trace=bool(int(os.environ.get("KERNEL_TRACE", "0"))),
    )
    _CACHE["last_result"] = res
    out = np.concatenate([r["out"] for r in res.results], axis=1)
    return out.reshape(B, N, D).astype(np.float32)
